# revision 43
# baseline (speedup 1.0000x reference)
"""GAT (2-layer, 4-head) Trainium2 kernel, 8-way row-parallel.

Strategy (scores kept transposed, [j=partition, i=free], i = this core's rows):
  exp(leaky_relu(s)) with s = f1_i + f2_j splits at s==0 into two rank-1
  factorable branches:
      s>=0: e^{f1_i} * e^{f2_j},   s<0: e^{0.01 f1_i} * e^{0.01 f2_j}
  so  alpha_num = A1 * (u1_i w1_j) + A2 * (u2_i w2_j)  with binary masks
      A1 = adjT * [f1_i + f2_j >= 0],  A2 = adjT - A1.
  Each mask is ONE fused scalar_tensor_tensor op; the masked softmax-matmul
  (alpha @ h, contraction over j) plus the softmax denominators come out of
  two fp16 PE matmuls per head against staged lhsT = [h * w | w].
  exp() only ever runs on O(N) vectors (host for layer 1, ScalarE for layer 2).
  Layer 1 h/f1/f2 are cheap O(N d^2) host precomputes (params x inputs only);
  everything O(N^2) runs on device. Layer 2 depends on layer-1 output: h2 is
  computed on device and exchanged via an 8-core AllGather ([512,18] fp32).
"""
import sys

for _p in ("/opt/trn_rl_repo", "/root/.axon_site/_ro/trn_rl_repo"):
    if _p not in sys.path:
        sys.path.insert(0, _p)

import numpy as np
import concourse.bass as bass
import concourse.bacc as bacc
import concourse.tile as tile
from concourse import mybir
from concourse.bass_utils import run_bass_kernel_spmd
from concourse.masks import make_identity

F16 = mybir.dt.float16
F32 = mybir.dt.float32

N = 4096
NODE_DIM = 256
D = 64            # hidden per head
NH = 4            # heads
C2 = 16           # n_classes
NCORE = 8
R = N // NCORE    # rows per core (512)
P = 128
NCHUNK = N // P   # 32 j-chunks
NEG = 0.01        # leaky slope
DL1 = D + 1       # 65: [h*w | w]
DL2 = C2 + 2      # 18: [h2 | f1 | f2]
IS_GE = mybir.AluOpType.is_ge
IS_LT = mybir.AluOpType.is_lt
MULT = mybir.AluOpType.mult
ADD = mybir.AluOpType.add
SUB = mybir.AluOpType.subtract
EXP = mybir.ActivationFunctionType.Exp

# fraction of A2 mask chunks computed on GPSIMD instead of DVE (load balance)
A2_GP = {k: (k * 7) % 32 < 14 for k in range(NCHUNK)}
# chunks whose p-compare runs on the idle ScalarE as Relu(Sign(s))
ACT_P = {k: (not A2_GP[k]) and (k * 7) % 32 < 22 for k in range(NCHUNK)}


def build_kernel(use_collective=True):
    nc = bacc.Bacc("TRN2", target_bir_lowering=False, debug=False, num_devices=NCORE)

    adjt_d = nc.dram_tensor("adjt", [NCHUNK, P, R], F16, kind="ExternalInput")
    lhs_d = nc.dram_tensor("lhs1", [NCHUNK, P, 3 * NH, DL1], F16, kind="ExternalInput")
    f1rep_d = nc.dram_tensor("f1rep", [P, NH, R], F16, kind="ExternalInput")
    negf2_d = nc.dram_tensor("negf2", [P, NH, NCHUNK], F32, kind="ExternalInput")
    urep_d = nc.dram_tensor("urep", [DL1, 2 * NH, R], F32, kind="ExternalInput")
    w2aug_d = nc.dram_tensor("w2aug", [2, P, DL2], F16, kind="ExternalInput")
    out_d = nc.dram_tensor("out", [C2 + 1, R], F32, kind="ExternalOutput")

    with tile.TileContext(nc) as tc:
        with (
            tc.tile_pool(name="const", bufs=1) as const,
            tc.tile_pool(name="mask", bufs=10) as mask,
            tc.tile_pool(name="comb", bufs=2) as comb,
            tc.tile_pool(name="small", bufs=2) as small,
            tc.tile_pool(name="l2", bufs=8) as l2p,
            tc.tile_pool(name="psum", bufs=2, space="PSUM") as psum,
            tc.tile_pool(name="psmall", bufs=2, space="PSUM") as psmall,
            tc.tile_pool(name="dram", bufs=1, space="DRAM") as dram,
        ):
            # ---------------- resident staged tensors ----------------
            adjt = const.tile([P, NCHUNK, R], F16)
            lhs = const.tile([P, NCHUNK, 3 * NH, DL1], F16)
            f1rep = const.tile([P, NH, R], F16)
            negf2 = const.tile([P, NH, NCHUNK], F32)
            urep = const.tile([DL1, 2 * NH, R], F32)
            w2aug = const.tile([P, 2, DL2], F16)
            # small control tensors first: head-0 masks need f1rep/negf2
            nc.sync.dma_start(out=f1rep, in_=f1rep_d[:, :, :])
            nc.sync.dma_start(out=negf2, in_=negf2_d[:, :, :])
            # piecewise loads: big enough to amortize the ~650ns/dma_start
            # HWDGE overhead, small enough that chunk-0 compute starts early
            PIECE = 2
            for k0 in range(0, NCHUNK, PIECE):
                nc.sync.dma_start(
                    out=adjt[:, k0:k0 + PIECE, :],
                    in_=adjt_d[k0:k0 + PIECE].rearrange("k p r -> p k r"))
                nc.sync.dma_start(
                    out=lhs[:, k0:k0 + PIECE, :, :],
                    in_=lhs_d[k0:k0 + PIECE].rearrange("k p v d -> p k v d"))
            nc.sync.dma_start(out=urep, in_=urep_d[:, :, :])
            for kk in range(2):
                nc.sync.dma_start(out=w2aug[:, kk, :], in_=w2aug_d[kk])
            ident = const.tile([32, 32], F32)
            make_identity(nc, ident)
            f2pos = const.tile([P, NH, NCHUNK], F32)
            nc.vector.tensor_scalar_mul(f2pos, negf2, -1.0)

            eluT = const.tile([P, 2, R], F16)      # layer-2 rhs, [feat, i]
            oth = [None] * NH                      # per-head numerators [65, R]

            # ---------------- layer 1 ----------------
            for h in range(NH):
                m1 = psum.tile([DL1, R], F32, tag="m1")
                m2 = psum.tile([DL1, R], F32, tag="m2")
                for k in range(NCHUNK):
                    pm = mask.tile([P, R], F16, tag="pm")
                    a1 = mask.tile([P, R], F16, tag="a1")
                    if ACT_P[k]:
                        # p = relu(sign(f1+f2)) on ScalarE (idle during L1);
                        # sign(0)=0 differs from is_ge only where s==0 exactly,
                        # where both branches carry identical weight anyway
                        qq = mask.tile([P, R], F16, tag="pm")
                        nc.scalar.activation(
                            out=qq, in_=f1rep[:, h, :],
                            func=mybir.ActivationFunctionType.Sign,
                            bias=f2pos[:, h, k:k + 1])
                        nc.scalar.activation(
                            out=pm, in_=qq,
                            func=mybir.ActivationFunctionType.Relu)
                    else:
                        nc.vector.tensor_scalar(
                            out=pm, in0=f1rep[:, h, :],
                            scalar1=negf2[:, h, k:k + 1],
                            scalar2=None, op0=IS_GE)
                    nc.vector.tensor_tensor(a1, pm, adjt[:, k, :], MULT)
                    nc.tensor.matmul(out=m1[:, :], lhsT=lhs[:, k, 3 * h, :], rhs=a1,
                                     start=(k == 0), stop=(k == NCHUNK - 1))
                    if A2_GP[k]:
                        # explicit A2 on the otherwise-idle Pool engine: 1 matmul
                        a2 = mask.tile([P, R], F16, tag="a2")
                        nc.gpsimd.tensor_tensor(
                            out=a2, in0=adjt[:, k, :], in1=a1, op=SUB)
                        nc.tensor.matmul(out=m2[:, :], lhsT=lhs[:, k, 3 * h + 1, :],
                                         rhs=a2,
                                         start=(k == 0), stop=(k == NCHUNK - 1))
                    else:
                        # M2 += hw2*adjT - hw2*A1 = hw2*A2 (no A2 materialized)
                        nc.tensor.matmul(out=m2[:, :], lhsT=lhs[:, k, 3 * h + 1, :],
                                         rhs=adjt[:, k, :],
                                         start=(k == 0), stop=False)
                        nc.tensor.matmul(out=m2[:, :], lhsT=lhs[:, k, 3 * h + 2, :],
                                         rhs=a1,
                                         start=False, stop=(k == NCHUNK - 1))
                # OT = u1 * M1 + u2 * M2   [65, R]
                t1 = comb.tile([DL1, R], F32, tag="t1")
                ot = comb.tile([DL1, R], F32, tag="ot", bufs=4)
                nc.vector.tensor_tensor(t1, m1, urep[:, 2 * h, :], MULT)
                nc.vector.tensor_tensor(ot, m2, urep[:, 2 * h + 1, :], MULT)
                nc.vector.tensor_tensor(ot, ot, t1, ADD)
                oth[h] = ot

            for h in range(NH):
                # per-head reciprocal: no cross-head barrier, so head h's elu
                # (and the layer-2 matmul) can start while later heads run
                rech = small.tile([1, R], F32, tag="rech")
                nc.vector.reciprocal(out=rech, in_=oth[h][D:DL1, :])
                recb = small.tile([D, R], F32, tag="recb")
                nc.gpsimd.partition_broadcast(out_ap=recb, in_ap=rech)
                oh = comb.tile([D, R], F32, tag="oh")
                nc.vector.tensor_tensor(oh, oth[h][0:D, :], recb, MULT)
                # elu(oh) -> eluT[(h%2)*64:..., h//2, :] fp16
                m0 = comb.tile([D, R], F32, tag="m0")
                e0 = comb.tile([D, R], F32, tag="e0")
                nc.vector.tensor_scalar_min(m0, oh, 0.0)
                nc.scalar.activation(out=e0, in_=m0, func=EXP)
                nc.vector.tensor_sub(oh, oh, m0)  # oh := relu part
                nc.vector.scalar_tensor_tensor(
                    out=eluT[(h % 2) * D:(h % 2) * D + D, h // 2, :],
                    in0=e0, scalar=-1.0, in1=oh, op0=ADD, op1=ADD)

            # ---------------- layer 2 ----------------
            # h2augT[18, R] = W2aug.T @ eluT
            h2t = psmall.tile([DL2, R], F32, tag="h2t")
            for kk in range(2):
                nc.tensor.matmul(out=h2t[:, :], lhsT=w2aug[:, kk, :], rhs=eluT[:, kk, :],
                                 start=(kk == 0), stop=(kk == 1))
            h2t_sb = comb.tile([DL2, R], F32, tag="h2tsb")
            nc.scalar.copy(h2t_sb, h2t)

            # f1_2 row + reps
            f12 = small.tile([1, R], F32, tag="f12")
            nc.sync.dma_start(out=f12, in_=h2t_sb[C2:C2 + 1, :])
            f12h = small.tile([1, R], F16, tag="f12h")
            nc.vector.tensor_copy(f12h, f12)
            f12rep = const.tile([P, R], F16)
            nc.gpsimd.partition_broadcast(out_ap=f12rep, in_ap=f12h)
            u1r = small.tile([1, R], F32, tag="u1r")
            u2r = small.tile([1, R], F32, tag="u2r")
            nc.scalar.activation(out=u1r, in_=f12, func=EXP)
            nc.scalar.activation(out=u2r, in_=f12, func=EXP, scale=NEG)
            u1rep = const.tile([C2 + 1, R], F32)
            u2rep = const.tile([C2 + 1, R], F32)
            nc.gpsimd.partition_broadcast(out_ap=u1rep, in_ap=u1r)
            nc.gpsimd.partition_broadcast(out_ap=u2rep, in_ap=u2r)

            # transpose h2augT -> h2aug_mine [512, 18] and AllGather
            h2m = comb.tile([P, 4, DL2], F32, tag="h2m")
            for q in range(4):
                tp = psmall.tile([P, DL2], F32, tag="tp")
                nc.tensor.transpose(out=tp, in_=h2t_sb[:, q * P:(q + 1) * P],
                                    identity=ident[0:DL2, 0:DL2])
                nc.scalar.copy(h2m[:, q, :], tp)
            agin = dram.tile([R, DL2], F32)
            agout = dram.tile([N, DL2], F32)
            nc.sync.dma_start(
                out=agin[:, :].rearrange("(q p) d -> p q d", q=4), in_=h2m)
            if use_collective:
                nc.gpsimd.collective_compute(
                    "AllGather", mybir.AluOpType.bypass,
                    replica_groups=[list(range(NCORE))],
                    ins=[agin.opt()], outs=[agout.opt()])
            else:  # timing-only stand-in (TimelineSim is single-core)
                for cc in range(NCORE):
                    nc.sync.dma_start(
                        out=agout[cc * R:(cc + 1) * R, :], in_=agin[:, :])

            m1 = psum.tile([DL2 - 1, R], F32, tag="m1")
            m2 = psum.tile([DL2 - 1, R], F32, tag="m2")
            h2all = const.tile([P, NCHUNK, DL2], F32)
            agr = agout[:, :].rearrange("(k p) d -> p k d", p=P)
            for k0 in range(0, NCHUNK, 8):
                nc.sync.dma_start(
                    out=h2all[:, k0:k0 + 8, :], in_=agr[:, k0:k0 + 8, :])
            for k in range(NCHUNK):
                h2c = h2all[:, k, :]
                w1c = l2p.tile([P, 1], F32, tag="w1c")
                w2c = l2p.tile([P, 1], F32, tag="w2c")
                ngc = l2p.tile([P, 1], F32, tag="ngc")
                nc.scalar.activation(out=w1c, in_=h2c[:, C2 + 1:C2 + 2], func=EXP)
                nc.scalar.activation(out=w2c, in_=h2c[:, C2 + 1:C2 + 2], func=EXP, scale=NEG)
                nc.scalar.mul(ngc, h2c[:, C2 + 1:C2 + 2], -1.0)
                hw1 = l2p.tile([P, C2 + 1], F16, tag="hw1")
                hw2 = l2p.tile([P, C2 + 1], F16, tag="hw2")
                hwn2 = l2p.tile([P, C2 + 1], F16, tag="hwn2")
                nc.vector.tensor_scalar_mul(hw1[:, 0:C2], h2c[:, 0:C2], w1c[:, 0:1])
                nc.scalar.copy(hw1[:, C2:C2 + 1], w1c)
                nc.vector.tensor_scalar_mul(hw2[:, 0:C2], h2c[:, 0:C2], w2c[:, 0:1])
                nc.scalar.copy(hw2[:, C2:C2 + 1], w2c)
                nc.scalar.mul(hwn2, hw2, -1.0)
                pm = mask.tile([P, R], F16, tag="pm")
                a1 = mask.tile([P, R], F16, tag="a1")
                nc.vector.tensor_scalar(
                    out=pm, in0=f12rep, scalar1=ngc[:, 0:1],
                    scalar2=None, op0=IS_GE)
                nc.vector.tensor_tensor(a1, pm, adjt[:, k, :], MULT)
                nc.tensor.matmul(out=m1[:, :], lhsT=hw1, rhs=a1,
                                 start=(k == 0), stop=(k == NCHUNK - 1))
                if A2_GP[k]:
                    a2 = mask.tile([P, R], F16, tag="a2")
                    nc.gpsimd.tensor_tensor(
                        out=a2, in0=adjt[:, k, :], in1=a1, op=SUB)
                    nc.tensor.matmul(out=m2[:, :], lhsT=hw2, rhs=a2,
                                     start=(k == 0), stop=(k == NCHUNK - 1))
                else:
                    nc.tensor.matmul(out=m2[:, :], lhsT=hw2, rhs=adjt[:, k, :],
                                     start=(k == 0), stop=False)
                    nc.tensor.matmul(out=m2[:, :], lhsT=hwn2, rhs=a1,
                                     start=False, stop=(k == NCHUNK - 1))

            t1 = comb.tile([C2 + 1, R], F32, tag="t1b")
            ot2 = comb.tile([C2 + 1, R], F32, tag="ot2")
            nc.vector.tensor_tensor(t1, m1, u1rep, MULT)
            nc.vector.tensor_tensor(ot2, m2, u2rep, MULT)
            nc.vector.tensor_tensor(ot2, ot2, t1, ADD)
            nc.sync.dma_start(out=out_d[:, :], in_=ot2)

    nc.compile()
    return nc


def host_prepare(x, adj_mat, W1, a1_1, a2_1, W2, a1_2, a2_2):
    """Build the per-core input maps (all fp32 math in numpy, fp16 staging)."""
    x = np.asarray(x, np.float32)
    adj = np.asarray(adj_mat)
    W1 = np.asarray(W1, np.float32)
    a1_1 = np.asarray(a1_1, np.float32)
    a2_1 = np.asarray(a2_1, np.float32)
    W2 = np.asarray(W2, np.float32)
    a1_2 = np.asarray(a1_2, np.float32)
    a2_2 = np.asarray(a2_2, np.float32)

    h = [x @ W1[k].T for k in range(NH)]              # [N, 64]
    f1 = [h[k] @ a1_1[k] for k in range(NH)]          # [N]
    f2 = [h[k] @ a2_1[k] for k in range(NH)]          # [N]

    # lhs1 [NCHUNK, P, 2*NH, DL1] fp16
    lhs = np.empty((N, 3 * NH, DL1), np.float32)
    for k in range(NH):
        w1 = np.exp(f2[k])
        w2 = np.exp(NEG * f2[k])
        lhs[:, 3 * k, :D] = h[k] * w1[:, None]
        lhs[:, 3 * k, D] = w1
        lhs[:, 3 * k + 1, :D] = h[k] * w2[:, None]
        lhs[:, 3 * k + 1, D] = w2
        lhs[:, 3 * k + 2, :] = -lhs[:, 3 * k + 1, :]
    lhs = lhs.reshape(NCHUNK, P, 3 * NH, DL1).astype(np.float16)

    # negf2 [P, NH, NCHUNK] f32 : negf2[p, h, k] = -f2[h][k*128+p]
    negf2 = np.empty((P, NH, NCHUNK), np.float32)
    for k in range(NH):
        negf2[:, k, :] = -f2[k].reshape(NCHUNK, P).T

    # W2aug [2, P, DL2] fp16
    w2aug = np.concatenate(
        [W2.T, (W2.T @ a1_2)[:, None], (W2.T @ a2_2)[:, None]], 1)
    w2aug = w2aug.reshape(2, P, DL2).astype(np.float16)

    adj16 = adj.astype(np.float16)
    in_maps = []
    for c in range(NCORE):
        rows = slice(c * R, (c + 1) * R)
        adjt = np.ascontiguousarray(adj16[rows, :].T).reshape(NCHUNK, P, R)
        f1rep = np.empty((P, NH, R), np.float16)
        urep = np.empty((DL1, 2 * NH, R), np.float32)
        for k in range(NH):
            f1rep[:, k, :] = f1[k][rows].astype(np.float16)[None, :]
            urep[:, 2 * k, :] = np.exp(f1[k][rows])[None, :]
            urep[:, 2 * k + 1, :] = np.exp(NEG * f1[k][rows])[None, :]
        in_maps.append({
            "adjt": adjt, "lhs1": lhs, "f1rep": f1rep, "negf2": negf2,
            "urep": urep, "w2aug": w2aug,
        })
    return in_maps


_CACHE = {}


def kernel(trace=False, **inputs):
    in_maps = host_prepare(**inputs)
    if "nc" not in _CACHE:
        _CACHE["nc"] = build_kernel()
    res = run_bass_kernel_spmd(
        _CACHE["nc"], in_maps, core_ids=list(range(NCORE)), trace=trace)
    outs = []
    for c in range(NCORE):
        o = res.results[c]["out"]                     # [17, R] f32
        outs.append((o[:C2, :] / o[C2:C2 + 1, :]).T)  # host division
    full = np.concatenate(outs, 0).astype(np.float32)
    if trace:
        return full, res
    return full
